# revision 13
# baseline (speedup 1.0000x reference)
"""Trainium2 Bass kernel for nn_Metrics (binary-classification metric sweep).

Strategy (8 NeuronCores, data-parallel over the 16M samples):
  Sharding: samples are permuted on the host by their (input) class label so
  each core receives a dense block of positives and a dense block of
  negatives (a pure input permutation — all metric math runs on device).
  Per core, samples are laid out [128 partitions x F].

  Stage A (device): u = 1/(1+exp(l0-l1)) via ACT Exp + DVE reciprocal
  (bitwise-IEEE), threshold-bin index k = rne(100*u + 0.5) via the 2^23
  rounding trick, packed to bf16 (integers <= 101, exact in bf16).

  Stage B (device): cumulative counts C_cls[m] = #{k <= m} for m = 0..100
  for each class region, as fused compare+accumulate passes split across
  the DVE (tensor_scalar is_le + accum, bf16 4x perf mode) and ACT
  (Sign trick: sum of sign(k - (m+.5)) = #above - #below, integer-exact
  in fp32). Each pass only scans the class region it needs — the
  class-split halves the all-pairs counting work.

  Host: aggregate per-core/per-partition partial counts, derive the
  confusion matrix, the 101-threshold precision/recall/specificity
  curves, average precision (exact within-bin hypergeometric expectation
  over the 102 score bins), and the loss mean.
"""

import math

import numpy as np

import concourse.bass as bass
import concourse.mybir as mybir
from concourse import tile
from concourse.bass_utils import run_bass_kernel_spmd

N = 16_000_000
NUM_TH = 101
NCORES = 8
P = 128
NTH = 101                     # thresholds m = 0..100 per class
ACC_COLS = 256                # acc output tensor width (>= 2*NTH)
CH = 1024                     # stage-A chunk width

# cost-model constants (ns per element per partition-lane) for engine balancing
_DVE_PASS_NS = 0.26           # bf16 4x tensor_scalar
_ACT_PASS_NS = 0.833
_DVE_STAGEA_NS = 2.1          # stage-A DVE work per element
_ACT_STAGEA_NS = 0.9

# u-pipeline: "sigmoid" = 1 ACT op (table ~40 ulp); "exp" = exact-ulp path
U_PIPELINE = "exp"

_F32 = mybir.dt.float32
_BF16 = mybir.dt.bfloat16

_wleg_ctr = [0]


def _legalize_waits(nc):
    """This walrus build encodes at most one sync-wait command per engine
    instruction; hoist extra waits onto inserted same-engine NOPs."""
    f = nc.m.functions[0]
    for blk in f.blocks:
        out = []
        changed = False
        for inst in blk.instructions:
            si = inst.sync_info
            waits = list(si.on_wait) if (si is not None and si.on_wait) else []
            n_upd = len(si.on_update) if (si is not None and si.on_update) else 0
            keep = 1 if n_upd <= 1 else 0
            if len(waits) > keep:
                hoist = waits[: len(waits) - keep] if keep else waits
                remain = waits[len(waits) - keep:] if keep else []
                for w in hoist:
                    _wleg_ctr[0] += 1
                    nop = mybir.InstNoOp(
                        name=f"WLEG-{_wleg_ctr[0]}",
                        engine=inst.engine,
                        ins=[],
                        outs=[],
                    )
                    nop.sync_info = mybir.SyncInfo(on_wait=[w], on_update=[])
                    out.append(nop)
                    changed = True
                inst.sync_info = mybir.SyncInfo(
                    on_wait=remain,
                    on_update=list(si.on_update) if si.on_update else [],
                )
            out.append(inst)
        if changed:
            blk.instructions = out
    return nc


def _assign_engines(fp: int, fn: int):
    """Greedy cost-balanced engine assignment for the 2*NTH stage-B passes.
    Returns a list of 'dve'/'act' per pass index (pass = cls*NTH + m)."""
    f_all = fp + fn
    dve_load = _DVE_STAGEA_NS * f_all
    act_load = _ACT_STAGEA_NS * f_all
    passes = [(0, m) for m in range(NTH)] + [(1, m) for m in range(NTH)]
    # interleave classes so both engines touch both regions evenly
    order = sorted(range(len(passes)), key=lambda i: (passes[i][1], passes[i][0]))
    eng = [""] * len(passes)
    for i in order:
        cls, _ = passes[i]
        sz = fp if cls == 0 else fn
        if dve_load + _DVE_PASS_NS * sz <= act_load + _ACT_PASS_NS * sz:
            eng[i] = "dve"
            dve_load += _DVE_PASS_NS * sz
        else:
            eng[i] = "act"
            act_load += _ACT_PASS_NS * sz
    return eng


_PROGRAM_CACHE = {}


def _build_program(fp: int, fn: int):
    """fp/fn: per-partition element counts of the positive/negative regions."""
    key = (fp, fn)
    if key in _PROGRAM_CACHE:
        return _PROGRAM_CACHE[key]

    f_all = fp + fn
    eng = _assign_engines(fp, fn)

    nc = bass.Bass()
    logits = nc.dram_tensor("logits", [P * f_all, 2], _F32, kind="ExternalInput")
    consts = nc.dram_tensor("consts", [P, ACC_COLS], _F32, kind="ExternalInput")
    acc_dve_out = nc.dram_tensor("acc_dve", [P, ACC_COLS], _F32, kind="ExternalOutput")
    acc_act_out = nc.dram_tensor("acc_act", [P, ACC_COLS], _F32, kind="ExternalOutput")

    lg_v = logits.ap().rearrange("(p f) c -> p f c", p=P)  # [128, f_all, 2]
    TWO23 = 8388608.0

    with tile.TileContext(nc) as tc:
        with (
            tc.tile_pool(name="fixed", bufs=1) as fixed,
            tc.tile_pool(name="work", bufs=3) as work,
        ):
            kbf = fixed.tile([P, f_all], _BF16)
            junk_d = fixed.tile([P, f_all], _BF16)
            junk_a = fixed.tile([P, f_all], _BF16)
            accd = fixed.tile([P, ACC_COLS], _F32)
            acca = fixed.tile([P, ACC_COLS], _F32)
            cst = fixed.tile([P, ACC_COLS], _F32)
            nc.sync.dma_start(out=cst[:], in_=consts.ap())

            # ---------------- Stage A ----------------
            for j0 in range(0, f_all, CH):
                j1 = min(j0 + CH, f_all)
                w = j1 - j0
                lg = work.tile([P, CH, 2], _F32, tag="lg")
                a = work.tile([P, CH], _F32, tag="a")
                b = work.tile([P, CH], _F32, tag="b")
                nc.sync.dma_start(out=lg[:, :w, :], in_=lg_v[:, j0:j1, :])
                if U_PIPELINE == "sigmoid":
                    # a = d = l1 - l0 ; u = sigmoid(d)
                    nc.vector.tensor_tensor(
                        a[:, :w], lg[:, :w, 1], lg[:, :w, 0],
                        mybir.AluOpType.subtract,
                    )
                    nc.scalar.activation(
                        b[:, :w], a[:, :w], mybir.ActivationFunctionType.Sigmoid,
                        bias=0.0, scale=1.0,
                    )
                else:
                    # a = -d = l0 - l1 ; e = exp(-d); s = e + 1; u = 1/s
                    nc.vector.tensor_tensor(
                        a[:, :w], lg[:, :w, 0], lg[:, :w, 1],
                        mybir.AluOpType.subtract,
                    )
                    nc.scalar.activation(
                        b[:, :w], a[:, :w], mybir.ActivationFunctionType.Exp,
                        bias=0.0, scale=1.0,
                    )
                    nc.vector.tensor_scalar(
                        b[:, :w], b[:, :w], 1.0, None, mybir.AluOpType.add
                    )
                    nc.vector.reciprocal(b[:, :w], b[:, :w])
                # x = u*100 + 0.5 ; x += 2^23 (RNE integer round; note
                # 2^23 + 0.5 is NOT fp32-representable, so two steps)
                nc.vector.tensor_scalar(
                    b[:, :w], b[:, :w], 100.0, 0.5, mybir.AluOpType.mult,
                    mybir.AluOpType.add,
                )
                nc.vector.tensor_scalar(
                    b[:, :w], b[:, :w], TWO23, None, mybir.AluOpType.add
                )
                # k = x - 2^23, converted to bf16 on the write
                nc.vector.tensor_scalar(
                    kbf[:, j0:j1], b[:, :w], TWO23, None, mybir.AluOpType.subtract
                )

            # ---------------- Stage B ----------------
            # pos-region passes first: that region's kbf is written first, so
            # engines start counting while stage A still fills the neg region
            for i in list(range(NTH)) + list(range(NTH, 2 * NTH)):
                cls, m = (0, i) if i < NTH else (1, i - NTH)
                reg = kbf[:, :fp] if cls == 0 else kbf[:, fp:f_all]
                if eng[i] == "dve":
                    nc.vector.tensor_scalar(
                        junk_d[:, :fp] if cls == 0 else junk_d[:, fp:f_all],
                        reg, float(m) + 0.5, None,
                        mybir.AluOpType.is_le, mybir.AluOpType.add,
                        accum_out=accd[:, i : i + 1],
                    )
                else:
                    nc.scalar.activation(
                        junk_a[:, :fp] if cls == 0 else junk_a[:, fp:f_all],
                        reg, mybir.ActivationFunctionType.Sign,
                        bias=cst[:, m : m + 1], scale=1.0,
                        accum_out=acca[:, i : i + 1],
                    )

            nc.sync.dma_start(out=acc_dve_out.ap(), in_=accd[:])
            nc.sync.dma_start(out=acc_act_out.ap(), in_=acca[:])

    _legalize_waits(nc)
    _PROGRAM_CACHE[key] = (nc, eng)
    return _PROGRAM_CACHE[key]


def _make_consts() -> np.ndarray:
    cst = np.zeros((P, ACC_COLS), np.float32)
    for m in range(NTH):
        cst[:, m] = -(m + 0.5)      # ACT Sign bias
    return cst


def _safe_div(n, d):
    n = np.asarray(n, np.float64)
    d = np.asarray(d, np.float64)
    return np.where(d > 0, n / np.maximum(d, 1.0), 0.0)


def _binned_ap(hist_all: np.ndarray, hist_pos: np.ndarray, total_pos: float) -> float:
    """Average precision from the 102-bin (k = 0..101) histogram, descending
    score order, exact expectation of the reference's within-bin ordering
    (labels are exchangeable within a score bin)."""
    n_tot = int(round(hist_all.sum()))
    H = np.zeros(n_tot + 1, np.float64)
    np.cumsum(1.0 / np.arange(1, n_tot + 1, dtype=np.float64), out=H[1:])

    ap_sum = 0.0
    Nab = 0.0
    Pab = 0.0
    for b in range(hist_all.shape[0] - 1, -1, -1):
        nb = float(hist_all[b])
        pb = float(hist_pos[b])
        if nb > 0 and pb > 0:
            if nb == 1:
                ap_sum += (Pab + 1.0) / (Nab + 1.0)
            else:
                beta = (pb - 1.0) / (nb - 1.0)
                alpha = Pab + 1.0 - beta
                hdiff = H[int(Nab + nb)] - H[int(Nab)]
                ap_sum += (pb / nb) * (beta * nb + (alpha - beta * Nab) * hdiff)
        Nab += nb
        Pab += pb
    return ap_sum / max(total_pos, 1.0)


_PAD_LOGIT = (0.0, 40.0)  # l1 - l0 = 40 -> u = 1.0 -> k = 100 (pad bin)


def kernel(pred_logits: np.ndarray, targets: np.ndarray, loss: np.ndarray):
    pred_logits = np.ascontiguousarray(np.asarray(pred_logits, np.float32))
    targets = np.asarray(targets)
    loss = np.asarray(loss, np.float32)
    n = pred_logits.shape[0]
    assert n == N

    # ---- host-side class-split sharding (pure input permutation) ----
    t = targets.astype(np.int64)
    pos_idx = np.flatnonzero(t == 1)
    neg_idx = np.flatnonzero(t != 1)
    p_total = int(pos_idx.shape[0])
    n_total = int(neg_idx.shape[0])

    fp = max(1, math.ceil(p_total / (NCORES * P)))
    fn = max(1, math.ceil(n_total / (NCORES * P)))
    f_all = fp + fn
    cap_p = NCORES * P * fp
    cap_n = NCORES * P * fn
    pad_p = cap_p - p_total
    pad_n = cap_n - n_total

    pos_lg = np.empty((cap_p, 2), np.float32)
    pos_lg[:p_total] = pred_logits[pos_idx]
    pos_lg[p_total:] = _PAD_LOGIT
    neg_lg = np.empty((cap_n, 2), np.float32)
    neg_lg[:n_total] = pred_logits[neg_idx]
    neg_lg[n_total:] = _PAD_LOGIT

    pos_lg = pos_lg.reshape(NCORES, P, fp, 2)
    neg_lg = neg_lg.reshape(NCORES, P, fn, 2)

    (nc, eng) = _build_program(fp, fn)
    consts = _make_consts()
    in_maps = []
    for c in range(NCORES):
        core_lg = np.concatenate([pos_lg[c], neg_lg[c]], axis=1)  # [P, f_all, 2]
        in_maps.append({
            "logits": np.ascontiguousarray(core_lg.reshape(P * f_all, 2)),
            "consts": consts,
        })
    res = run_bass_kernel_spmd(nc, in_maps, list(range(NCORES)))

    # ---- decode counts: C_cls[m] = #{k <= m} over the class, m = 0..100 ----
    accd = np.zeros(ACC_COLS, np.float64)
    acca = np.zeros(ACC_COLS, np.float64)
    for r in res.results:
        accd += np.asarray(r["acc_dve"], np.float64).sum(axis=0)
        acca += np.asarray(r["acc_act"], np.float64).sum(axis=0)

    C = np.zeros(2 * NTH, np.float64)
    for i in range(2 * NTH):
        cls = 0 if i < NTH else 1
        region_total = float(cap_p if cls == 0 else cap_n)
        if eng[i] == "dve":
            C[i] = accd[i]
        else:
            C[i] = (region_total - acca[i]) / 2.0
    C_pos = C[:NTH].copy()
    C_neg = C[NTH:].copy()
    # padding lands in bin k=100 — remove it from the m=100 cumulative count
    C_pos[100] -= pad_p
    C_neg[100] -= pad_n

    # ---- final metric math (host, fp64) ----
    total_pos = float(p_total)

    hist_pos = np.diff(np.concatenate([[0.0], C_pos, [total_pos]]))   # k=0..101
    hist_neg = np.diff(np.concatenate([[0.0], C_neg, [float(n_total)]]))
    hist_all = hist_pos + hist_neg

    cum_all = np.cumsum(hist_all)
    cum_pos = np.cumsum(hist_pos)

    idx_f = cum_all[:NUM_TH]
    fn_t = cum_pos[:NUM_TH]
    tp_t = total_pos - fn_t
    tn_t = idx_f - fn_t
    fp_t = (float(n) - idx_f) - tp_t

    prec_t = _safe_div(tp_t, tp_t + fp_t)
    rec_t = _safe_div(tp_t, tp_t + fn_t)
    spec_t = _safe_div(tn_t, tn_t + fp_t)
    ths = (np.arange(NUM_TH, dtype=np.float32) * np.float32(0.01)).astype(np.float32)

    fn0 = float(cum_pos[50])
    tp0 = total_pos - fn0
    tn0 = float(cum_all[50]) - fn0
    fp0 = (float(n) - float(cum_all[50])) - tp0
    confmat = np.array([[tn0, fp0], [fn0, tp0]], np.float64)

    recall = float(_safe_div(tp0, tp0 + fn0))
    precision = float(_safe_div(tp0, tp0 + fp0))
    specificity = float(_safe_div(tn0, tn0 + fp0))
    accuracy = (tp0 + tn0) / float(n)

    map_ = _binned_ap(hist_all, hist_pos, total_pos)
    loss_out = float(np.asarray(loss, np.float64).mean())

    prts = (
        prec_t.astype(np.float32),
        rec_t.astype(np.float32),
        ths,
        spec_t.astype(np.float32),
    )
    return (
        confmat.astype(np.float32),
        np.float32(map_),
        prts,
        np.float32(recall),
        np.float32(precision),
        np.float32(specificity),
        np.float32(accuracy),
        np.float32(loss_out),
    )
